# revision 15
# baseline (speedup 1.0000x reference)
"""Segment-max kernel for Trainium2 (8 NeuronCores, SPMD).

Computes out[s] = max over points p with batch_indices[p] == s of
encoded_feats[p], for S = B*patch_num segments (empty segments -> 0),
returning shape (B, patch_num, D).

Strategy: batch_indices is sorted, so each segment is a contiguous row
range of encoded_feats. The host splits every non-empty segment into
windows: full windows of exactly L points plus one tail window, with
tails bucketed by width (multiples of 8) and clamp-padded to their
bucket width by replicating the last point (harmless for max). Each
(window, feature) pair is an independent fixed-width stream; per width
bucket, all streams of one core are laid out row-major into a
[128, W_b*b] region so all 128 vector lanes and 16 DMA ports do useful
work. The device streams the concatenated regions through SBUF with
large pipelined DMAs and runs one 3-D tensor_reduce(max)
[128, ch, b] -> [128, ch] per tile. The host finishes by regrouping
window results per segment (argsort + np.maximum.reduceat).
"""

import sys

if "/opt/trn_rl_repo" not in sys.path:
    sys.path.insert(0, "/opt/trn_rl_repo")

import numpy as np
import ml_dtypes

BF16 = ml_dtypes.bfloat16
NCORES = 8
P = 128            # SBUF partitions
TILE_COLS = 8960   # free-dim columns per SBUF load tile (17.5 KiB/partition bf16)
N_BUFS = 6
MAX_W = 32000      # result tile [128, Wtot] must fit in SBUF

_LAST = {}
_PROGRAM_CACHE = {}


def _choose_L(counts, seg_core, D):
    """Pick full-window width L (multiple of 8) minimizing streamed bytes
    with bucketed tails: cost per window ~ bucket_width + 1 (out word)."""
    maxcnt = int(counts.max()) if counts.size else 8
    cap = max(8, min(((maxcnt + 7) // 8) * 8, 4096))
    cands = np.arange(8, cap + 8, 8)
    nz = counts > 0
    c = counts[nz]
    core = seg_core[nz]
    qpad = 128 // np.gcd(128, D)
    best = None
    for L in cands:
        L = int(L)
        nbuck = L // 8
        nfull = c // L
        tail = c - nfull * L  # 0..L-1
        # true layout cost: per-bucket counts are maxed over cores and
        # rounded up to qpad, full windows are bucket L (index nbuck-1)
        cnt_cb = np.zeros((NCORES, nbuck), dtype=np.int64)
        np.add.at(cnt_cb, (core, np.full(len(c), nbuck - 1)), nfull)
        ht = tail > 0
        np.add.at(cnt_cb, (core[ht], (tail[ht] + 7) // 8 - 1), 1)
        nsub_b = cnt_cb.max(axis=0)
        nsub_b = ((nsub_b + qpad - 1) // qpad) * qpad
        bw = np.arange(1, nbuck + 1) * 8
        cost = int((nsub_b * (bw + 1)).sum())
        if best is None or cost < best[0]:
            best = (cost, L)
    assert best is not None
    return best[1]


def _build_program(regions, repeat=1):
    """regions: list of (bucket_width b, W_b, nt_shapes_b). g columns and o
    columns are the concatenation of regions in order."""
    key = (tuple((b, W, tuple(s)) for b, W, s in regions), repeat)
    if key in _PROGRAM_CACHE:
        return _PROGRAM_CACHE[key]

    import concourse.tile as tile
    from concourse import bacc, mybir

    gcols = sum(W * b for b, W, _ in regions)
    ocols = sum(W for _, W, _ in regions)
    nc = bacc.Bacc("TRN2", target_bir_lowering=False, debug=False,
                   num_devices=NCORES)
    g = nc.dram_tensor("g", [P, gcols], mybir.dt.bfloat16,
                       kind="ExternalInput").ap()
    o = nc.dram_tensor("o", [P, ocols], mybir.dt.bfloat16,
                       kind="ExternalOutput").ap()

    ntiles = sum(len(s) for _, _, s in regions)
    flush_every = 3
    chmax = max(min(max(1, TILE_COLS // b), W) for b, W, _ in regions)
    colmax = max(
        min(max(1, TILE_COLS // b), W) * b for b, W, _ in regions
    )
    # tile layout: [0, colmax) data, [colmax, colmax+smax) ping-pong
    # scratch, [colmax+smax, +chmax) tmp column for the width-3 finish
    smax = colmax // 2 + chmax + 8
    tmp0 = colmax + smax
    tot = tmp0 + chmax

    with tile.TileContext(nc) as tc:
        with (
            tc.tile_pool(name="inp", bufs=N_BUFS) as pool,
            tc.tile_pool(name="res", bufs=1) as opool,
        ):
            ot = opool.tile([P, ocols], mybir.dt.bfloat16)

            def reduce_tile(tl, ch, b, oslice):
                """Pairwise tensor_max tree [P, ch, b] -> oslice [P, ch].
                All wide ops keep operands innermost-packed bf16 so the
                DVE 2x_1p mode applies (0.5 cyc/elem vs 1.0 for
                tensor_reduce, which supports no fast modes)."""
                view = lambda off, w: tl[:, off : off + ch * w].rearrange(
                    "p (c l) -> p c l", l=w)
                off, w = 0, b
                ping = 0
                while w > 3:
                    h, odd = divmod(w, 2)
                    noff = colmax if ping == 0 else 0
                    dst = view(noff, h + odd)
                    x = view(off, w)
                    nc.vector.tensor_max(dst[:, :, :h], x[:, :, :h],
                                         x[:, :, h : 2 * h])
                    if odd:
                        nc.vector.tensor_copy(dst[:, :, h], x[:, :, 2 * h])
                    off, w, ping = noff, h + odd, 1 - ping
                x = view(off, w)
                if w == 3:
                    t = tl[:, tmp0 : tmp0 + ch]
                    nc.vector.tensor_max(t, x[:, :, 0], x[:, :, 1])
                    nc.vector.tensor_max(oslice, t, x[:, :, 2])
                elif w == 2:
                    nc.vector.tensor_max(oslice, x[:, :, 0], x[:, :, 1])
                else:
                    nc.vector.tensor_copy(oslice, x[:, :, 0])

            def body(_i=None):
                goff = 0   # input column offset
                c0 = 0     # output column offset
                f0 = 0
                i = 0
                for b, W, nt_shapes in regions:
                    for ch in nt_shapes:
                        tl = pool.tile([P, tot], mybir.dt.bfloat16,
                                       tag="ld")
                        eng = nc.sync if i % 2 == 0 else nc.scalar
                        eng.dma_start(tl[:, : ch * b],
                                      g[:, goff : goff + ch * b])
                        reduce_tile(tl, ch, b, ot[:, c0 : c0 + ch])
                        goff += ch * b
                        c0 += ch
                        i += 1
                        if i % flush_every == 0 or i == ntiles:
                            nc.scalar.dma_start(o[:, f0:c0], ot[:, f0:c0])
                            f0 = c0

            if repeat == 1:
                body()
            else:
                with tc.For_i(0, repeat, 1) as _i:
                    body(_i)

    nc.compile()
    _PROGRAM_CACHE[key] = nc
    return nc


def _tile_shapes(b, W, split_last=False):
    ch_full = min(max(1, TILE_COLS // b), W)
    shapes = []
    left = W
    while left > 0:
        shapes.append(min(ch_full, left))
        left -= shapes[-1]
    if split_last and shapes and shapes[-1] > 8:
        c = shapes.pop()
        h = c // 2
        # shrink the pipeline drain: the final DMA+tree pair is half-size
        shapes.extend([c - h, h])
    return shapes


def _prepare(encoded_feats, batch_indices, S):
    feats = np.ascontiguousarray(encoded_feats, dtype=np.float32)
    idx = np.asarray(batch_indices)
    if idx.size > 1 and not np.all(idx[1:] >= idx[:-1]):
        order = np.argsort(idx, kind="stable")
        idx = idx[order]
        feats = feats[order]
    M, D = feats.shape

    feats = feats.astype(BF16)

    st = np.searchsorted(idx, np.arange(S + 1))
    counts = np.diff(st).astype(np.int64)
    seg_lo = (np.arange(NCORES + 1) * S) // NCORES
    seg_core = np.repeat(np.arange(NCORES), np.diff(seg_lo))

    L = _choose_L(counts, seg_core, D)
    qpad = 128 // np.gcd(128, D)
    buckets = list(range(8, L + 1, 8))

    # per-core window tables (seg-ordered), bucket assignment
    percore = []
    for d in range(NCORES):
        segs = np.arange(seg_lo[d], seg_lo[d + 1])
        segs = segs[counts[segs] > 0]
        cnt = counts[segs]
        nfull = cnt // L
        tail = cnt - nfull * L
        ns = nfull + (tail > 0)
        p_total = int(ns.sum())
        run_starts = np.zeros(len(segs), dtype=np.int64)
        if len(segs) > 1:
            run_starts[1:] = np.cumsum(ns)[:-1]
        wseg = np.repeat(np.arange(len(segs)), ns)          # local seg id
        k = np.arange(p_total) - run_starts[wseg]
        wstart = st[segs[wseg]] + k * L
        wwidth = np.minimum(cnt[wseg] - k * L, L)           # 1..L
        wbucket = ((wwidth + 7) // 8) * 8                   # 8..L
        percore.append(dict(segs=segs, ns=ns, wseg=wseg, wstart=wstart,
                            wwidth=wwidth, wbucket=wbucket,
                            p_total=p_total))

    # global per-bucket counts (max over cores, rounded to qpad)
    NSUB_b = {}
    for b in buckets:
        n = max(int((pc["wbucket"] == b).sum()) for pc in percore)
        n = ((n + qpad - 1) // qpad) * qpad
        NSUB_b[b] = n
    total_w = sum(NSUB_b.values())
    assert total_w * D // P <= MAX_W, "output tile too large"

    nzb = [b for b in buckets if NSUB_b[b] > 0]
    regions = [(b, NSUB_b[b] * D // P,
                _tile_shapes(b, NSUB_b[b] * D // P, split_last=(b == nzb[-1])))
               for b in nzb]

    cores = []
    for d in range(NCORES):
        pc = percore[d]
        Gparts = []
        # per-core window order after bucketing (for postprocess)
        ord_parts = []
        for b in buckets:
            nb = NSUB_b[b]
            if nb == 0:
                continue
            sel = np.nonzero(pc["wbucket"] == b)[0]
            starts = np.zeros(nb, dtype=np.int64)
            widths = np.ones(nb, dtype=np.int64)
            starts[: len(sel)] = pc["wstart"][sel]
            widths[: len(sel)] = pc["wwidth"][sel]
            offs = np.arange(b, dtype=np.int64)
            rowidx = starts[:, None] + np.minimum(offs[None, :],
                                                  (widths - 1)[:, None])
            gath = feats[rowidx.ravel()].reshape(nb, b, D)
            W_b = nb * D // P
            Gparts.append(
                np.ascontiguousarray(gath.transpose(0, 2, 1))
                .reshape(P, W_b * b)
            )
            ord_parts.append((sel, len(sel), nb))
        G = np.concatenate(Gparts, axis=1) if Gparts else np.zeros(
            (P, 0), BF16)
        cores.append(dict(G=G, pc=pc, ord_parts=ord_parts))

    meta = dict(L=L, D=D, S=S, counts=counts, regions=regions,
                NSUB_b=NSUB_b, cores=cores,
                total_w=total_w)
    return meta


def _postprocess(results, meta):
    S, D = meta["S"], meta["D"]
    out = np.zeros((S, D), dtype=np.float32)
    for d, core in enumerate(meta["cores"]):
        pc = core["pc"]
        if pc["p_total"] == 0:
            continue
        o = np.asarray(results[d]["o"]).astype(np.float32)  # (P, sum W_b)
        # reassemble window results into original seg-ordered positions;
        # each region is independently row-major [P, W_b] -> (NSUB_b, D)
        res = np.empty((pc["p_total"], D), dtype=np.float32)
        coff = 0
        for (b, W_b, _), (sel, nreal, nb) in zip(meta["regions"],
                                                 core["ord_parts"]):
            rb = np.ascontiguousarray(o[:, coff : coff + W_b]).reshape(nb, D)
            res[sel] = rb[:nreal]
            coff += W_b
        run_starts = np.zeros(len(pc["segs"]), dtype=np.int64)
        if len(pc["segs"]) > 1:
            run_starts[1:] = np.cumsum(pc["ns"])[:-1]
        segmax = np.maximum.reduceat(res, run_starts, axis=0)
        out[pc["segs"]] = segmax
    return out


def kernel(encoded_feats, batch_indices, B, patch_num):
    from concourse.bass_utils import run_bass_kernel_spmd

    B = int(B)
    patch_num = int(patch_num)
    S = B * patch_num
    meta = _prepare(encoded_feats, batch_indices, S)

    nc = _build_program(meta["regions"], repeat=1)
    in_maps = [{"g": core["G"]} for core in meta["cores"]]
    res = run_bass_kernel_spmd(nc, in_maps, list(range(NCORES)))

    _LAST.clear()
    _LAST.update(meta=meta, nc=nc, in_maps=in_maps, results=res)

    out = _postprocess(res.results, meta)
    return out.reshape(B, patch_num, meta["D"])



# revision 20
# speedup vs baseline: 1.0914x; 1.0914x over previous
"""Segment-max kernel for Trainium2 (8 NeuronCores, SPMD).

Computes out[s] = max over points p with batch_indices[p] == s of
encoded_feats[p], for S = B*patch_num segments (empty segments -> 0),
returning shape (B, patch_num, D).

Strategy: batch_indices is sorted, so each segment is a contiguous row
range of encoded_feats. The host splits every non-empty segment into
windows: full windows of exactly L points plus one tail window, with
tails bucketed by width (multiples of 8) and clamp-padded to their
bucket width by replicating the last point (harmless for max). Each
(window, feature) pair is an independent fixed-width stream; per width
bucket, all streams of one core are laid out row-major into a
[128, W_b*b] region so all 128 vector lanes and 16 DMA ports do useful
work. The device streams the concatenated regions through SBUF with
large pipelined DMAs and runs one 3-D tensor_reduce(max)
[128, ch, b] -> [128, ch] per tile. The host finishes by regrouping
window results per segment (argsort + np.maximum.reduceat).
"""

import sys

if "/opt/trn_rl_repo" not in sys.path:
    sys.path.insert(0, "/opt/trn_rl_repo")

import numpy as np
import ml_dtypes

BF16 = ml_dtypes.bfloat16
NCORES = 8
P = 128            # SBUF partitions
TILE_COLS = 7168   # free-dim columns per SBUF load tile (14 KiB/partition bf16)
N_BUFS = 8
MAX_W = 32000      # result tile [128, Wtot] must fit in SBUF

_LAST = {}
_PROGRAM_CACHE = {}


def _choose_L(counts, seg_core, D):
    """Pick full-window width L (multiple of 8) minimizing streamed bytes
    with bucketed tails: cost per window ~ bucket_width + 1 (out word)."""
    maxcnt = int(counts.max()) if counts.size else 8
    cap = max(8, min(((maxcnt + 7) // 8) * 8, 4096))
    cands = np.arange(8, cap + 8, 8)
    nz = counts > 0
    c = counts[nz]
    core = seg_core[nz]
    qpad = 128 // np.gcd(128, D)
    best = None
    for L in cands:
        L = int(L)
        nbuck = L // 8
        nfull = c // L
        tail = c - nfull * L  # 0..L-1
        # true layout cost: per-bucket counts are maxed over cores and
        # rounded up to qpad, full windows are bucket L (index nbuck-1)
        cnt_cb = np.zeros((NCORES, nbuck), dtype=np.int64)
        np.add.at(cnt_cb, (core, np.full(len(c), nbuck - 1)), nfull)
        ht = tail > 0
        np.add.at(cnt_cb, (core[ht], (tail[ht] + 7) // 8 - 1), 1)
        nsub_b = cnt_cb.max(axis=0)
        nsub_b = ((nsub_b + qpad - 1) // qpad) * qpad
        bw = np.arange(1, nbuck + 1) * 8
        cost = int((nsub_b * (bw + 1)).sum())
        if best is None or cost < best[0]:
            best = (cost, L)
    assert best is not None
    return best[1]


def _build_program(regions, repeat=1):
    """regions: list of (bucket_width b, W_b, nt_shapes_b). g columns and o
    columns are the concatenation of regions in order."""
    key = (tuple((b, W, tuple(s)) for b, W, s in regions), repeat)
    if key in _PROGRAM_CACHE:
        return _PROGRAM_CACHE[key]

    import concourse.tile as tile
    from concourse import bacc, mybir

    gcols = sum(W * b for b, W, _ in regions)
    ocols = sum(W for _, W, _ in regions)
    nc = bacc.Bacc("TRN2", target_bir_lowering=False, debug=False,
                   num_devices=NCORES)
    g = nc.dram_tensor("g", [P, gcols], mybir.dt.bfloat16,
                       kind="ExternalInput").ap()
    o = nc.dram_tensor("o", [P, ocols], mybir.dt.bfloat16,
                       kind="ExternalOutput").ap()

    ntiles = sum(len(s) for _, _, s in regions)
    flush_every = max(4, (ntiles + 4) // 5)
    chmax = max(min(max(1, TILE_COLS // b), W) for b, W, _ in regions)
    colmax = max(
        min(max(1, TILE_COLS // b), W) * b for b, W, _ in regions
    )
    # tile layout: [0, colmax) data, [colmax, colmax+smax) ping-pong
    # scratch, [colmax+smax, +chmax) tmp column for the width-3 finish
    smax = colmax // 2 + chmax + 8
    tmp0 = colmax + smax
    tot = tmp0 + chmax

    with tile.TileContext(nc) as tc:
        with (
            tc.tile_pool(name="inp", bufs=N_BUFS) as pool,
            tc.tile_pool(name="res", bufs=1) as opool,
        ):
            ot = opool.tile([P, ocols], mybir.dt.bfloat16)

            def reduce_tile(tl, ch, b, oslice):
                """Pairwise tensor_max tree [P, ch, b] -> oslice [P, ch].
                All wide ops keep operands innermost-packed bf16 so the
                DVE 2x_1p mode applies (0.5 cyc/elem vs 1.0 for
                tensor_reduce, which supports no fast modes)."""
                view = lambda off, w: tl[:, off : off + ch * w].rearrange(
                    "p (c l) -> p c l", l=w)
                off, w = 0, b
                ping = 0
                while w > 3:
                    h, odd = divmod(w, 2)
                    noff = colmax if ping == 0 else 0
                    dst = view(noff, h + odd)
                    x = view(off, w)
                    nc.vector.tensor_max(dst[:, :, :h], x[:, :, :h],
                                         x[:, :, h : 2 * h])
                    if odd:
                        nc.vector.tensor_copy(dst[:, :, h], x[:, :, 2 * h])
                    off, w, ping = noff, h + odd, 1 - ping
                x = view(off, w)
                if w == 3:
                    t = tl[:, tmp0 : tmp0 + ch]
                    nc.vector.tensor_max(t, x[:, :, 0], x[:, :, 1])
                    nc.vector.tensor_max(oslice, t, x[:, :, 2])
                elif w == 2:
                    nc.vector.tensor_max(oslice, x[:, :, 0], x[:, :, 1])
                else:
                    nc.vector.tensor_copy(oslice, x[:, :, 0])

            def body(_i=None):
                goff = 0   # input column offset
                c0 = 0     # output column offset
                i = 0
                for b, W, nt_shapes in regions:
                    for ch in nt_shapes:
                        tl = pool.tile([P, tot], mybir.dt.bfloat16,
                                       tag="ld")
                        eng = nc.sync if i % 2 == 0 else nc.scalar
                        eng.dma_start(tl[:, : ch * b],
                                      g[:, goff : goff + ch * b])
                        reduce_tile(tl, ch, b, ot[:, c0 : c0 + ch])
                        goff += ch * b
                        c0 += ch
                        i += 1
                # single end-of-stream store: mid-stream flushes wait on
                # the DVE and head-of-line-block the loads queued behind
                # them on the shared HWDGE queue
                nc.scalar.dma_start(o[:, 0:ocols], ot[:, 0:ocols])

            if repeat == 1:
                body()
            else:
                with tc.For_i(0, repeat, 1) as _i:
                    body(_i)

    nc.compile()
    _PROGRAM_CACHE[key] = nc
    return nc


def _tile_shapes(b, W, split_last=False):
    ch_full = min(max(1, TILE_COLS // b), W)
    shapes = []
    left = W
    while left > 0:
        shapes.append(min(ch_full, left))
        left -= shapes[-1]
    if split_last and shapes and shapes[-1] > 8:
        c = shapes.pop()
        h = c // 2
        # shrink the pipeline drain: the final DMA+tree pair is half-size
        shapes.extend([c - h, h])
    return shapes


def _prepare(encoded_feats, batch_indices, S):
    feats = np.ascontiguousarray(encoded_feats, dtype=np.float32)
    idx = np.asarray(batch_indices)
    if idx.size > 1 and not np.all(idx[1:] >= idx[:-1]):
        order = np.argsort(idx, kind="stable")
        idx = idx[order]
        feats = feats[order]
    M, D = feats.shape

    feats = feats.astype(BF16)

    st = np.searchsorted(idx, np.arange(S + 1))
    counts = np.diff(st).astype(np.int64)
    seg_lo = (np.arange(NCORES + 1) * S) // NCORES
    seg_core = np.repeat(np.arange(NCORES), np.diff(seg_lo))

    L = _choose_L(counts, seg_core, D)
    qpad = 128 // np.gcd(128, D)
    buckets = list(range(8, L + 1, 8))

    # per-core window tables (seg-ordered), bucket assignment
    percore = []
    for d in range(NCORES):
        segs = np.arange(seg_lo[d], seg_lo[d + 1])
        segs = segs[counts[segs] > 0]
        cnt = counts[segs]
        nfull = cnt // L
        tail = cnt - nfull * L
        ns = nfull + (tail > 0)
        p_total = int(ns.sum())
        run_starts = np.zeros(len(segs), dtype=np.int64)
        if len(segs) > 1:
            run_starts[1:] = np.cumsum(ns)[:-1]
        wseg = np.repeat(np.arange(len(segs)), ns)          # local seg id
        k = np.arange(p_total) - run_starts[wseg]
        wstart = st[segs[wseg]] + k * L
        wwidth = np.minimum(cnt[wseg] - k * L, L)           # 1..L
        wbucket = ((wwidth + 7) // 8) * 8                   # 8..L
        percore.append(dict(segs=segs, ns=ns, wseg=wseg, wstart=wstart,
                            wwidth=wwidth, wbucket=wbucket,
                            p_total=p_total))

    # global per-bucket counts (max over cores, rounded to qpad)
    NSUB_b = {}
    for b in buckets:
        n = max(int((pc["wbucket"] == b).sum()) for pc in percore)
        n = ((n + qpad - 1) // qpad) * qpad
        NSUB_b[b] = n
    total_w = sum(NSUB_b.values())
    assert total_w * D // P <= MAX_W, "output tile too large"

    nzb = [b for b in buckets if NSUB_b[b] > 0]
    regions = [(b, NSUB_b[b] * D // P,
                _tile_shapes(b, NSUB_b[b] * D // P, split_last=(b == nzb[-1])))
               for b in nzb]

    cores = []
    for d in range(NCORES):
        pc = percore[d]
        Gparts = []
        # per-core window order after bucketing (for postprocess)
        ord_parts = []
        for b in buckets:
            nb = NSUB_b[b]
            if nb == 0:
                continue
            sel = np.nonzero(pc["wbucket"] == b)[0]
            starts = np.zeros(nb, dtype=np.int64)
            widths = np.ones(nb, dtype=np.int64)
            starts[: len(sel)] = pc["wstart"][sel]
            widths[: len(sel)] = pc["wwidth"][sel]
            offs = np.arange(b, dtype=np.int64)
            rowidx = starts[:, None] + np.minimum(offs[None, :],
                                                  (widths - 1)[:, None])
            gath = feats[rowidx.ravel()].reshape(nb, b, D)
            W_b = nb * D // P
            Gparts.append(
                np.ascontiguousarray(gath.transpose(0, 2, 1))
                .reshape(P, W_b * b)
            )
            ord_parts.append((sel, len(sel), nb))
        G = np.concatenate(Gparts, axis=1) if Gparts else np.zeros(
            (P, 0), BF16)
        cores.append(dict(G=G, pc=pc, ord_parts=ord_parts))

    meta = dict(L=L, D=D, S=S, counts=counts, regions=regions,
                NSUB_b=NSUB_b, cores=cores,
                total_w=total_w)
    return meta


def _postprocess(results, meta):
    S, D = meta["S"], meta["D"]
    out = np.zeros((S, D), dtype=np.float32)
    for d, core in enumerate(meta["cores"]):
        pc = core["pc"]
        if pc["p_total"] == 0:
            continue
        o = np.asarray(results[d]["o"]).astype(np.float32)  # (P, sum W_b)
        # reassemble window results into original seg-ordered positions;
        # each region is independently row-major [P, W_b] -> (NSUB_b, D)
        res = np.empty((pc["p_total"], D), dtype=np.float32)
        coff = 0
        for (b, W_b, _), (sel, nreal, nb) in zip(meta["regions"],
                                                 core["ord_parts"]):
            rb = np.ascontiguousarray(o[:, coff : coff + W_b]).reshape(nb, D)
            res[sel] = rb[:nreal]
            coff += W_b
        run_starts = np.zeros(len(pc["segs"]), dtype=np.int64)
        if len(pc["segs"]) > 1:
            run_starts[1:] = np.cumsum(pc["ns"])[:-1]
        segmax = np.maximum.reduceat(res, run_starts, axis=0)
        out[pc["segs"]] = segmax
    return out


def kernel(encoded_feats, batch_indices, B, patch_num):
    from concourse.bass_utils import run_bass_kernel_spmd

    B = int(B)
    patch_num = int(patch_num)
    S = B * patch_num
    meta = _prepare(encoded_feats, batch_indices, S)

    nc = _build_program(meta["regions"], repeat=1)
    in_maps = [{"g": core["G"]} for core in meta["cores"]]
    res = run_bass_kernel_spmd(nc, in_maps, list(range(NCORES)))

    _LAST.clear()
    _LAST.update(meta=meta, nc=nc, in_maps=in_maps, results=res)

    out = _postprocess(res.results, meta)
    return out.reshape(B, patch_num, meta["D"])

